# revision 22
# baseline (speedup 1.0000x reference)
"""Trainium2 Bass kernel for nn_CapXLayer (CapsNet-style layer).

Sharding: data-parallel over batch. 8 batches -> 8 NeuronCores, one batch
per core. All parameters replicated. Full inputs in, full output out.

Per-core dataflow (one batch, CH-layout [channels, pixels], px chunks of 512,
processed stage-major in groups of 4 chunks so every engine queue always has
independent cross-chunk work):

  conv:    relu(x) once -> conv1 (1x1 grouped) -> conv2 (3x3 grouped SAME,
           9 taps x 2 halves accumulated in PSUM, zero-padded [128,10,66]
           bands) -> u_pt[th] tiles (j-layout, bias folded at PSUM->SBUF copy)
  layouts: q = 16*oc + od            (s tiles; ALSO the output channel order,
                                      so the tail needs no permutation)
           j = 32*(oc>>1) + 16*(oc&1) + 4*icg + odw   (u_pt rows, th=(t,h),
                                      ic = 4h+icg, od = 4t+odw)
           r = 16*oc + ic            ("scattered" rows for per-(ic,oc) values:
                                      nsq/g/c/b/d; rows 16oc+8.. are unused
                                      garbage kept finite by zero mask columns)
           all replications (c -> cb[h], s16 -> drep[t]) are quadrant-local
           stream_shuffles under these layouts.
  routing: squash factors g = n/((0.5+n)*sqrt(n+1e-6)) computed with DVE
           pow/divide (no ACT Sqrt => the sigmoid act table stays resident,
           a single table load for the whole kernel)
  tail:    spatial capsule attention in natural layout + residual + store
"""

import numpy as np

import concourse.bass as bass
import concourse.bacc as bacc
import concourse.tile as tile
import concourse.mybir as mybir
from concourse.bass_utils import run_bass_kernel_spmd

F32 = mybir.dt.float32
F32R = mybir.dt.float32r
BF16 = mybir.dt.bfloat16
AF = mybir.ActivationFunctionType
OP = mybir.AluOpType

IC, IND, MID, OC, OD = 8, 16, 32, 8, 16
B, H, W = 8, 64, 64
PX = H * W            # 4096
CS = 512              # pixels per chunk
NCH = PX // CS        # 8 chunks
G = 4                 # chunks in flight (stage-major group)

TAPS = [(dy, dx) for dy in (-1, 0, 1) for dx in (-1, 0, 1)]

# j-layout helpers (u_pt rows): j = 32*(oc>>1) + 16*(oc&1) + 4*icg + odw
_j = np.arange(128)
J_OC = 2 * (_j >> 5) + ((_j >> 4) & 1)
J_ICG = (_j >> 2) & 3
J_ODW = _j & 3
# q-layout (s rows): q = 16*oc + od
_q = np.arange(128)
Q_OC = _q >> 4
Q_OD = _q & 15

# stream_shuffle masks (32-entry, per-quadrant; quadrant = oc>>1 everywhere)
_i = np.arange(32)
_i_oc1 = _i >> 4
_i_icg = (_i >> 2) & 3
_i_odw = _i & 3
MASK_CB = [list(16 * _i_oc1 + 4 * h + _i_icg) for h in range(2)]
MASK_DREP = [list(16 * _i_oc1 + 4 * t + _i_odw) for t in range(4)]


# ---------------------------------------------------------------- host prep
def _prep_consts(w1, b1, w2, b2, w3, b3, attn_w, attn_b):
    """Precompute matmul-ready weight layouts and constant matrices."""
    import ml_dtypes
    c = {}
    # conv1 lhsT: [128, 128]; rows 64h..64h+63 hold half h's lhsT so the
    # lhsT slice shares its base partition with the rhs x-slice
    w1L = np.zeros((128, 128), np.float32)
    for h in range(2):
        for g in range(4):
            gg = h * 4 + g
            w1L[64 * h + g * 16:64 * h + (g + 1) * 16,
                g * 32:(g + 1) * 32] = w1[gg * 32:(gg + 1) * 32, :, 0, 0].T
    c["w1L"] = w1L
    # conv2 lhsT: [128, 2, 9, 128]
    w2L = np.zeros((128, 2, 9, 128), np.float32)
    for h in range(2):
        for t, (dy, dx) in enumerate(TAPS):
            for g in range(4):
                gg = h * 4 + g
                w2L[g * 32:(g + 1) * 32, h, t, g * 32:(g + 1) * 32] = \
                    w2[gg * 32:(gg + 1) * 32, :, dy + 1, dx + 1].T
    c["w2L"] = w2L
    # biases as per-partition columns
    c["b1s"] = np.stack([b1[0:128], b1[128:256]], axis=1).astype(np.float32)
    c["b2s"] = np.stack([b2[0:128], b2[128:256]], axis=1).astype(np.float32)
    # conv3 lhsT (u_pt production): w3P[k, th, j], k = 32*icg + mid
    w3P = np.zeros((128, 8, 128), np.float32)
    b3P = np.zeros((128, 8), np.float32)
    for th in range(8):
        t, h = th >> 1, th & 1
        ch_full = (4 * h + J_ICG) * 128 + 16 * J_OC + 4 * t + J_ODW
        for j in range(128):
            k = J_ICG[j] * 32 + np.arange(MID)
            w3P[k, th, j] = w3[ch_full[j], :, 0, 0]
        b3P[:, th] = b3[ch_full]
    c["w3P"] = w3P
    c["b3P"] = b3P
    # accum masks: product rows j -> s rows q = 16*oc + 4t + odw, summing
    # (icg, h) via the 8-matmul PSUM accumulation. accMh folds iter-1's 0.5.
    accM = np.zeros((128, 4, 128), np.float32)
    for t in range(4):
        accM[_j, t, 16 * J_OC + 4 * t + J_ODW] = 1.0
    c["accM"] = accM.astype(ml_dtypes.bfloat16)
    c["accMh"] = (0.5 * accM).astype(ml_dtypes.bfloat16)
    # reduce masks: product rows j -> scattered rows r = 16*oc + ic,
    # summing (t, odw) via th accumulation
    redD = np.zeros((128, 8, 128), np.float32)
    for th in range(8):
        h = th & 1
        redD[_j, th, 16 * J_OC + 4 * h + J_ICG] = 1.0
    c["redD"] = redD.astype(ml_dtypes.bfloat16)
    # ns reduce: s rows q -> scattered rows 16*oc + ic, replicated over ic
    onesB = np.zeros((128, 128), np.float32)
    for ic in range(IC):
        onesB[_q, 16 * Q_OC + ic] = 1.0
    c["onesB"] = onesB.astype(ml_dtypes.bfloat16)
    # attention tail constants (avg packed [64,512], partition = 8c+oc)
    onesA = np.zeros((128, 8), np.float32)
    onesA[_q, Q_OC] = 1.0
    c["onesA"] = onesA
    sel64 = np.zeros((64, 8), np.float32)        # sum over chunk blocks
    rep64 = np.zeros((8, 64), np.float32)        # replicate [8,1] -> [64,1]
    for cc in range(NCH):
        for ocv in range(OC):
            sel64[cc * 8 + ocv, ocv] = 1.0
            rep64[ocv, cc * 8 + ocv] = 1.0
    c["sel64"] = sel64
    c["rep64"] = rep64
    # selrep[:, c, :]: [64, 8, 128] -- replicate rows 8c..8c+7 (the chunk's
    # [8,CS] sigmoid block) over od into q rows
    selrep = np.zeros((64, 8, 128), np.float32)
    for cc in range(NCH):
        selrep[cc * 8 + Q_OC, cc, _q] = 1.0
    c["selrep"] = selrep.astype(ml_dtypes.bfloat16)
    c["aw"] = attn_w.reshape(1, OC).astype(np.float32).copy()
    c["ab"] = attn_b.reshape(1, OC).astype(np.float32).copy()
    c["zpad"] = np.zeros((128, 66), np.float32)
    return c


F32_CONSTS = {"b1s", "b2s", "b3P", "aw", "ab", "sel64", "rep64"}
BF16_CONSTS = {"accM", "accMh", "redD", "onesB", "selrep"}

CONST_SHAPES = {
    "w1L": [128, 128], "w2L": [128, 2, 9, 128], "w3P": [128, 8, 128],
    "b1s": [128, 2], "b2s": [128, 2], "b3P": [128, 8],
    "accM": [128, 4, 128], "accMh": [128, 4, 128],
    "redD": [128, 8, 128], "onesB": [128, 128],
    "onesA": [128, 8], "sel64": [64, 8], "rep64": [8, 64],
    "selrep": [64, 8, 128], "aw": [1, 8], "ab": [1, 8], "zpad": [128, 66],
}


def build_nc(num_devices=8, stage=99):
    nc = bacc.Bacc("TRN2", target_bir_lowering=False, debug=False,
                   num_devices=num_devices)

    io = {}
    io["x"] = nc.dram_tensor("x", [128, PX], F32R, kind="ExternalInput").ap()
    for name, shp in CONST_SHAPES.items():
        dt = (F32 if name in F32_CONSTS else
              BF16 if name in BF16_CONSTS else F32R)
        io[name] = nc.dram_tensor(name, shp, dt, kind="ExternalInput").ap()
    out_dram = nc.dram_tensor("out", [128, PX], F32, kind="ExternalOutput").ap()

    with tile.TileContext(nc) as tc:
        _body(tc, io, out_dram, stage)
    nc.compile()
    return nc


def _body(tc, io, out_dram, stage=99):
    nc = tc.nc

    import contextlib
    ctx = contextlib.ExitStack()
    with ctx:
        consts = ctx.enter_context(tc.tile_pool(name="consts", bufs=1))
        cs_t = {}
        for name, shp in CONST_SHAPES.items():
            dt = (F32 if name in F32_CONSTS else
                  BF16 if name in BF16_CONSTS else F32R)
            t = consts.tile(shp, dt, name=name, tag=name)
            nc.sync.dma_start(out=t[:], in_=io[name])
            cs_t[name] = t

        persist = ctx.enter_context(tc.tile_pool(name="persist", bufs=1))
        x_sb = persist.tile([128, PX], F32R, name="x_sb", tag="x_sb")
        nc.sync.dma_start(out=x_sb[:], in_=io["x"])
        sf_sb = persist.tile([128, PX], F32, name="sf", tag="sf")
        avg64 = persist.tile([64, CS], F32, name="avg64", tag="avg64")
        zp = cs_t["zpad"]
        cb_eps = persist.tile([128, 1], F32, name="cb_eps", tag="cb_eps")
        nc.vector.memset(cb_eps[:], 1e-6)

        # ------------------------------------------------ pools
        ph2ps = contextlib.ExitStack()
        hb = ph2ps.enter_context(tc.tile_pool(name="hb", bufs=2))
        h2p = ph2ps.enter_context(tc.tile_pool(name="h2p", bufs=G))
        upp = ph2ps.enter_context(tc.tile_pool(name="upp", bufs=1))
        sm = ph2ps.enter_context(tc.tile_pool(name="sm", bufs=1))
        smt = ph2ps.enter_context(tc.tile_pool(name="smt", bufs=2))
        scr = ph2ps.enter_context(tc.tile_pool(name="scr", bufs=2))
        pcv = ph2ps.enter_context(
            tc.tile_pool(name="pcv", bufs=3, space="PSUM"))
        pred = ph2ps.enter_context(
            tc.tile_pool(name="pred", bufs=2, space="PSUM"))
        psa = ph2ps.enter_context(
            tc.tile_pool(name="psa", bufs=2, space="PSUM"))

        # TT engine round-robin: most to DVE, every third-ish to Pool
        rr = {"i": 0}

        def tt_eng():
            rr["i"] += 1
            return nc.gpsimd if rr["i"] % 4 == 0 else nc.vector

        # ------------------------------------------------ conv stages
        def conv1_band(c):
            r_lo = max(8 * c - 1, 0)
            r_hi = min(8 * c + 9, H)
            n = r_hi - r_lo
            idx_lo = r_lo - (8 * c - 1)
            rxb = hb.tile([128, 10 * W], F32R, name="rxb", tag="rxb")
            nc.scalar.activation(out=rxb[:, 0:n * W],
                                 in_=x_sb[:, r_lo * W:r_hi * W], func=AF.Relu)
            h1b = [hb.tile([128, 10, 66], F32R, name=f"h1b{h}",
                           tag=f"h1b{h}") for h in range(2)]
            for h in range(2):
                # zero the padding columns (and edge rows at image boundary)
                nc.sync.dma_start(
                    out=h1b[h][:, :, 0:1],
                    in_=zp[:, 0:10].rearrange("p (a b) -> p a b", b=1))
                nc.sync.dma_start(
                    out=h1b[h][:, :, 65:66],
                    in_=zp[:, 0:10].rearrange("p (a b) -> p a b", b=1))
                if c == 0:
                    nc.sync.dma_start(out=h1b[h][:, 0, :], in_=zp[:, 0:66])
                if c == NCH - 1:
                    nc.sync.dma_start(out=h1b[h][:, 9, :], in_=zp[:, 0:66])
                k1 = n // 2
                for ro, k in ((0, k1), (k1, n - k1)):
                    ps = pcv.tile([128, CS], F32, name="cvps", tag="cvps")
                    nc.tensor.matmul(
                        ps[:, 0:k * W],
                        cs_t["w1L"][h * 64:(h + 1) * 64, :],
                        rxb[h * 64:(h + 1) * 64, ro * W:(ro + k) * W],
                        start=True, stop=True)
                    nc.scalar.activation(
                        out=h1b[h][:, idx_lo + ro:idx_lo + ro + k, 1:65],
                        in_=ps[:, 0:k * W].rearrange("p (a b) -> p a b", a=k),
                        func=AF.Relu, bias=cs_t["b1s"][:, h:h + 1], scale=1.0)
            return h1b

        def conv2_band(c, h1b):
            h2b = [h2p.tile([128, CS], F32R, name=f"h2b{h}", tag=f"h2b{h}")
                   for h in range(2)]
            for h in range(2):
                ps = pcv.tile([128, CS], F32, name="cvps", tag="cvps")
                for t, (dy, dx) in enumerate(TAPS):
                    nc.tensor.matmul(
                        ps[:],
                        cs_t["w2L"][:, h, t, :],
                        h1b[h][:, 1 + dy:9 + dy, 1 + dx:65 + dx],
                        start=(t == 0), stop=(t == len(TAPS) - 1))
                nc.scalar.activation(
                    out=h2b[h][:], in_=ps[:],
                    func=AF.Relu, bias=cs_t["b2s"][:, h:h + 1], scale=1.0)
            return h2b

        def conv3_upt(sl, h2b):
            """u_pt[th] = w3P[:,th,:]^T @ h2b[h] + b3P (bias folded at the
            PSUM->SBUF copy). Relu'd conv2 output in, j-layout bf16 out."""
            u_pt = []
            for th in range(8):
                ps = pcv.tile([128, CS], F32, name="cvps", tag="cvps")
                nc.tensor.matmul(ps[:], cs_t["w3P"][:, th, :],
                                 h2b[th & 1][:], start=True, stop=True)
                u_t = upp.tile([128, CS], BF16, name=f"u{th}",
                               tag=f"u{sl}_{th}")
                nc.scalar.activation(out=u_t[:], in_=ps[:], func=AF.Identity,
                                     bias=cs_t["b3P"][:, th:th + 1], scale=1.0)
                u_pt.append(u_t)
            return u_pt

        # ------------------------------------------------ routing pieces
        def g_chain(n_sb, gpool, gtag):
            """g = n / ((0.5+n) * sqrt(n+1e-6)) -> bf16 [128, CS].
            ACT Sqrt (stage-batched to amortize table loads) + DVE STT/div."""
            rt = smt.tile([128, CS], F32, name="g_rt", tag="g_rt")
            nc.scalar.activation(out=rt[:], in_=n_sb[:], func=AF.Sqrt,
                                 bias=cb_eps[:], scale=1.0)
            den = smt.tile([128, CS], F32, name="g_den", tag="g_den")
            nc.vector.scalar_tensor_tensor(out=den[:], in0=n_sb[:], scalar=0.5,
                                           in1=rt[:], op0=OP.add, op1=OP.mult)
            rg = smt.tile([128, CS], F32, name="g_rg", tag="g_rg")
            nc.vector.reciprocal_approx_fast(out=rg[:], in_=den[:])
            g_t = gpool.tile([128, CS], BF16, name="g_g", tag=gtag)
            nc.vector.tensor_tensor(out=g_t[:], in0=n_sb[:], in1=rg[:],
                                    op=OP.mult)
            return g_t

        def accum_pass(u_pt, cT, masks, s_ps):
            """s_ps[q] = sum_(icg,h) cb*u_pt; cb[h] = quadrant shuffle of the
            scattered c tile."""
            cb = []
            for h in range(2):
                cbt = scr.tile([128, CS], BF16, name=f"cb{h}", tag=f"cb{h}")
                nc.vector.stream_shuffle(out=cbt[:], in_=cT[:],
                                         mask=MASK_CB[h])
                cb.append(cbt)
            for th in range(8):
                t, h = th >> 1, th & 1
                p_t = scr.tile([128, CS], BF16, name="pp", tag="pp")
                tt_eng().tensor_tensor(out=p_t[:], in0=u_pt[th][:],
                                       in1=cb[h][:], op=OP.mult)
                nc.tensor.matmul(s_ps[:], masks[:, t, :], p_t[:],
                                 start=(th == 0), stop=(th == 7))

        def d_pass(u_pt, s16, red_ps):
            """red_ps[16oc+ic] = sum_od u_pt*srep; drep[t] = quadrant
            shuffle of s16 (q-natural)."""
            drep = []
            for t in range(4):
                dt_ = scr.tile([128, CS], BF16, name=f"dr{t}", tag=f"dr{t}")
                nc.vector.stream_shuffle(out=dt_[:], in_=s16[:],
                                         mask=MASK_DREP[t])
                drep.append(dt_)
            for th in range(8):
                t = th >> 1
                q_t = scr.tile([128, CS], BF16, name="qq", tag="qq")
                tt_eng().tensor_tensor(out=q_t[:], in0=u_pt[th][:],
                                       in1=drep[t][:], op=OP.mult)
                nc.tensor.matmul(red_ps[:], cs_t["redD"][:, th, :], q_t[:],
                                 start=(th == 0), stop=(th == 7))

        # ------------------------------------------------ chunk state
        st = [dict() for _ in range(G)]

        def s_nsq(c, sl):
            u_pt = st[sl]["u"]
            nsq_ps = pred.tile([128, CS], F32, name="red", tag="red")
            for th in range(8):
                sq_t = scr.tile([128, CS], BF16, name="sq", tag="sq")
                tt_eng().tensor_tensor(out=sq_t[:], in0=u_pt[th][:],
                                       in1=u_pt[th][:], op=OP.mult)
                nc.tensor.matmul(nsq_ps[:], cs_t["redD"][:, th, :], sq_t[:],
                                 start=(th == 0), stop=(th == 7))
            n_sb = smt.tile([128, CS], F32, name="nsq", tag="nsq")
            nc.scalar.copy(out=n_sb[:], in_=nsq_ps[:])
            st[sl]["g_u"] = g_chain(n_sb, sm, f"gu_{sl}")

        def s_iter_a(c, sl, it):
            """Sqrt-table half of a routing iteration: accum -> s16 -> ns ->
            g -> d -> b update. No Sigmoid here so the act table is stable
            across the whole 4-chunk stage."""
            u_pt = st[sl]["u"]
            g_u = st[sl]["g_u"]
            if it == 1:
                cT, masks = g_u, cs_t["accMh"]
            else:
                cT, masks = st[sl]["ct2"], cs_t["accM"]
            s_ps = psa.tile([128, CS], F32, name="sacc", tag="sacc")
            accum_pass(u_pt, cT, masks, s_ps)
            s16 = sm.tile([128, CS], BF16, name="s16", tag=f"s16_{sl}")
            nc.scalar.copy(out=s16[:], in_=s_ps[:])
            # squash factor of s
            ssq = scr.tile([128, CS], BF16, name="ssq", tag="ssq")
            nc.scalar.activation(out=ssq[:], in_=s16[:], func=AF.Square)
            ns_ps = pred.tile([128, CS], F32, name="red", tag="red")
            nc.tensor.matmul(ns_ps[:], cs_t["onesB"][:], ssq[:],
                             start=True, stop=True)
            nsb = smt.tile([128, CS], F32, name="nsb", tag="nsb")
            nc.scalar.copy(out=nsb[:], in_=ns_ps[:])
            g_i = g_chain(nsb, smt, "g_i")
            # d = sum_od u*s ; b += d*g_u*g_i
            d_ps = pred.tile([128, CS], F32, name="red", tag="red")
            d_pass(u_pt, s16, d_ps)
            gg = smt.tile([128, CS], BF16, name="gg", tag="gg")
            nc.vector.tensor_tensor(out=gg[:], in0=g_i[:], in1=g_u[:],
                                    op=OP.mult)
            if it == 1:
                b2 = sm.tile([128, CS], F32, name="b2", tag=f"b2_{sl}")
                nc.vector.tensor_tensor(out=b2[:], in0=d_ps[:], in1=gg[:],
                                        op=OP.mult)
                st[sl]["b2"] = b2
            else:
                tb = smt.tile([128, CS], F32, name="tb", tag="tb")
                nc.vector.tensor_tensor(out=tb[:], in0=d_ps[:], in1=gg[:],
                                        op=OP.mult)
                b3 = sm.tile([128, CS], F32, name="b3", tag=f"b3_{sl}")
                nc.vector.tensor_tensor(out=b3[:], in0=tb[:],
                                        in1=st[sl]["b2"][:], op=OP.add)
                st[sl]["b3"] = b3

        def s_iter_b(c, sl, it):
            """Sigmoid-table half: c = sigmoid(b) (+ct2 for iter 1)."""
            g_u = st[sl]["g_u"]
            if it == 1:
                c2 = smt.tile([128, CS], BF16, name="c2", tag="c2")
                nc.scalar.activation(out=c2[:], in_=st[sl]["b2"][:],
                                     func=AF.Sigmoid)
                ct2 = sm.tile([128, CS], BF16, name="ct2", tag=f"ct2_{sl}")
                nc.vector.tensor_tensor(out=ct2[:], in0=c2[:], in1=g_u[:],
                                        op=OP.mult)
                st[sl]["ct2"] = ct2
            else:
                c3 = sm.tile([128, CS], BF16, name="c3", tag=f"c3_{sl}")
                nc.scalar.activation(out=c3[:], in_=st[sl]["b3"][:],
                                     func=AF.Sigmoid)
                st[sl]["c3"] = c3

        def s_final(c, sl):
            csl = slice(c * CS, (c + 1) * CS)
            sf_ps = psa.tile([128, CS], F32, name="sacc", tag="sacc")
            accum_pass(st[sl]["u"], st[sl]["c3"], cs_t["accM"], sf_ps)
            nc.scalar.copy(out=sf_sb[:, csl], in_=sf_ps[:])

        # ------------------------------------------------ main loop
        for si in range(NCH // G):
            cs = list(range(si * G, (si + 1) * G))
            # conv1 one chunk ahead of conv2 so PE never waits on the ACT
            # relu copies of the same chunk
            h1b_cur = {cs[0]: conv1_band(cs[0])}
            h2bs = {}
            for k, c in enumerate(cs):
                if k + 1 < G:
                    h1b_cur[cs[k + 1]] = conv1_band(cs[k + 1])
                h2bs[c] = conv2_band(c, h1b_cur.pop(c))
            for c in cs:
                sl = c % G
                st[sl]["u"] = conv3_upt(sl, h2bs[c])
            for c in cs:
                s_nsq(c, c % G)
            for it in (1, 2):
                for c in cs:
                    s_iter_a(c, c % G, it)
                for c in cs:
                    s_iter_b(c, c % G, it)
            for c in cs:
                s_final(c, c % G)

        if stage <= 4:
            ph2ps.close()
            nc.sync.dma_start(out=out_dram, in_=sf_sb[:])
            return

        # ---------------- tail: spatial capsule attention ----------------
        ph2ps.close()
        tailp = ctx.enter_context(tc.tile_pool(name="tailp", bufs=2))
        tt = ctx.enter_context(tc.tile_pool(name="tt", bufs=1))
        dramp = ctx.enter_context(tc.tile_pool(name="dramp", bufs=1,
                                               space="DRAM"))
        ppt = ctx.enter_context(tc.tile_pool(name="ppt", bufs=2, space="PSUM"))

        mh = tt.tile([128, 1], F32, name="mh", tag="mh")
        nc.vector.reduce_sum(out=mh[:], in_=sf_sb[:], axis=mybir.AxisListType.X)
        nc.scalar.mul(mh[:], mh[:], 1.0 / PX)

        # avg packed [64, CS]: partition 8c+oc holds chunk c's avg row oc
        for c in range(NCH):
            csl = slice(c * CS, (c + 1) * CS)
            scrc = tailp.tile([128, CS], F32R, name="p", tag="p")
            nc.vector.tensor_scalar(out=scrc[:], in0=sf_sb[:, csl],
                                    scalar1=mh[:], scalar2=None, op0=OP.mult)
            av_ps = ppt.tile([8, CS], F32, name="avgc", tag="avgc")
            nc.tensor.matmul(av_ps[:], cs_t["onesA"][:],
                             scrc[:], start=True, stop=True)
            # compute engines need 32-aligned start partitions; bounce via
            # SBUF and let DMA scatter to partition 8c
            avst = tailp.tile([8, CS], F32, name="avst", tag="avst")
            nc.scalar.copy(out=avst[:], in_=av_ps[:])
            nc.sync.dma_start(out=avg64[8 * c:8 * c + 8, :], in_=avst[:])

        rowsum = tt.tile([64, 1], F32, name="rowsum", tag="rowsum")
        nc.vector.reduce_sum(out=rowsum[:], in_=avg64[:],
                             axis=mybir.AxisListType.X)
        # gather the 64 per-(chunk,oc) row sums onto one partition, reduce
        # the chunk axis there, and broadcast back — avoids tiny PE matmuls
        rowsT = tt.tile([1, 64], F32, name="rowsT", tag="rowsT")
        nc.sync.dma_start(out=rowsT[:], in_=rowsum[:])
        m_row = tt.tile([1, 8], F32, name="m_row", tag="m_row")
        nc.vector.reduce_sum(
            out=m_row[:],
            in_=bass.AP(tensor=rowsT.tensor, offset=rowsT.offset,
                        ap=[[64, 1], [1, 8], [8, 8]]),
            axis=mybir.AxisListType.X)
        nc.scalar.mul(m_row[:], m_row[:], 1.0 / PX)
        mrow_d = dramp.tile([1, 8], F32, name="mrow_d", tag="mrow_d")
        nc.sync.dma_start(out=mrow_d[:], in_=m_row[:])
        m64 = tt.tile([64, 1], F32, name="m64", tag="m64")
        nc.sync.dma_start(
            out=m64[:],
            in_=bass.AP(tensor=mrow_d.tensor, offset=mrow_d.offset,
                        ap=[[0, 8], [1, 8]]))
        cen = tt.tile([64, CS], F32, name="cen", tag="cen")
        nc.vector.tensor_scalar(out=cen[:], in0=avg64[:], scalar1=m64[:],
                                scalar2=None, op0=OP.subtract)
        vjunk = tt.tile([64, CS], F32, name="vjunk", tag="vjunk")
        nc.vector.tensor_tensor(out=vjunk[:], in0=cen[:], in1=cen[:],
                                op=OP.mult)
        v64 = tt.tile([64, 1], F32, name="v64", tag="v64")
        nc.vector.reduce_sum(out=v64[:], in_=vjunk[:],
                             axis=mybir.AxisListType.X)
        vT = tt.tile([1, 64], F32, name="vT", tag="vT")
        nc.sync.dma_start(out=vT[:], in_=v64[:])
        var8 = tt.tile([1, 8], F32, name="var8", tag="var8")
        nc.vector.reduce_sum(
            out=var8[:],
            in_=bass.AP(tensor=vT.tensor, offset=vT.offset,
                        ap=[[64, 1], [1, 8], [8, 8]]),
            axis=mybir.AxisListType.X)
        cb_eps = tt.tile([1, 1], F32, name="cb_eps", tag="cb_eps")
        nc.vector.memset(cb_eps[:], 1e-6)
        sd8 = tt.tile([1, 8], F32, name="sd8", tag="sd8")
        nc.scalar.activation(out=sd8[:], in_=var8[:], func=AF.Sqrt,
                             bias=0.0, scale=1.0 / (PX - 1))
        nc.scalar.activation(out=sd8[:], in_=sd8[:], func=AF.Identity,
                             bias=cb_eps[:1], scale=1.0)
        rsd8 = tt.tile([1, 8], F32, name="rsd8", tag="rsd8")
        nc.vector.reciprocal(out=rsd8[:], in_=sd8[:])
        rsdw8 = tt.tile([1, 8], F32, name="rsdw8", tag="rsdw8")
        nc.vector.tensor_tensor(out=rsdw8[:], in0=rsd8[:], in1=cs_t["aw"][:],
                                op=OP.mult)
        rsdw_d = dramp.tile([1, 8], F32, name="rsdw_d", tag="rsdw_d")
        nc.sync.dma_start(out=rsdw_d[:], in_=rsdw8[:])
        rw64 = tt.tile([64, 1], F32, name="rw64", tag="rw64")
        nc.sync.dma_start(
            out=rw64[:],
            in_=bass.AP(tensor=rsdw_d.tensor, offset=rsdw_d.offset,
                        ap=[[0, 8], [1, 8]]))
        ab64 = tt.tile([64, 1], F32, name="ab64", tag="ab64")
        nc.sync.dma_start(
            out=ab64[:],
            in_=bass.AP(tensor=io["ab"].tensor, offset=io["ab"].offset,
                        ap=[[0, 8], [1, 8]]))
        t2 = tt.tile([64, CS], F32, name="t2", tag="t2")
        nc.vector.tensor_scalar(out=t2[:], in0=cen[:], scalar1=rw64[:],
                                scalar2=ab64[:], op0=OP.mult, op1=OP.add)
        sig = tt.tile([64, CS], BF16, name="sig", tag="sig")
        nc.scalar.activation(out=sig[:], in_=t2[:], func=AF.Sigmoid)

        for c in range(NCH):
            csl = slice(c * CS, (c + 1) * CS)
            srep = ppt.tile([128, CS], F32, name="srep", tag="srep")
            nc.tensor.matmul(srep[:], cs_t["selrep"][:, c, :],
                             sig[:], start=True, stop=True)
            o1 = tailp.tile([128, CS], F32, name="o1", tag="o1")
            nc.vector.tensor_tensor(out=o1[:], in0=srep[:], in1=sf_sb[:, csl],
                                    op=OP.mult)
            o2 = tailp.tile([128, CS], F32, name="o2", tag="o2")
            eng = nc.gpsimd if c % 2 == 0 else nc.vector
            eng.tensor_tensor(out=o2[:], in0=o1[:], in1=x_sb[:, csl],
                              op=OP.add)
            nc.sync.dma_start(out=out_dram[:, c * CS:(c + 1) * CS], in_=o2[:])


# ---------------------------------------------------------------- dispatch
_NC_CACHE = {}


def _get_nc():
    if "nc" not in _NC_CACHE:
        _NC_CACHE["nc"] = build_nc()
    return _NC_CACHE["nc"]


def kernel(x, w1, b1, w2, b2, w3, b3, attn_w, attn_b):
    x = np.ascontiguousarray(np.asarray(x, dtype=np.float32))
    consts = _prep_consts(
        np.asarray(w1, np.float32), np.asarray(b1, np.float32),
        np.asarray(w2, np.float32), np.asarray(b2, np.float32),
        np.asarray(w3, np.float32), np.asarray(b3, np.float32),
        np.asarray(attn_w, np.float32), np.asarray(attn_b, np.float32))
    consts = {k: np.ascontiguousarray(v) for k, v in consts.items()}

    nc = _get_nc()
    in_maps = []
    for b in range(B):
        m = {"x": x[b].reshape(128, PX).copy()}
        m.update(consts)
        in_maps.append(m)
    res = run_bass_kernel_spmd(nc, in_maps, core_ids=list(range(B)))
    out = np.zeros((B, 128, H, W), np.float32)
    for b in range(B):
        out[b] = res.results[b]["out"].reshape(128, H, W)
    return out
